# revision 50
# baseline (speedup 1.0000x reference)
"""BranchingAttention (ViewFormer) Trainium2 Bass kernel.

Problem: two token streams x0 (trunk) / x1, fused qkv projection
(w_attn packs v|q|k), block-causal multi-end attention:
  query token t in block i of branch e attends ALL tokens of trunk
  blocks j < i plus causally (u <= t) its own branch's block i,
joint softmax, out projection.  Returns (out0, out1).

Sharding (8 cores): data-parallel over batch (B=2) x tensor-parallel
over 4 head-groups of 3 heads.  Each core computes BOTH branches for
its 3 heads and emits partial projections; the host sums the 4 head
-group partials per (branch, batch) and adds b_proj.

Per-core device kernel (uniform SPMD program, bf16 matmuls):
  - inputs are host-pre-transposed (xT: [d, tok]) and head-sliced
  - qkv: psum[grp] = sum_dchunk Wg[dc].T @ xT[dc] -> QV sbuf (bf16)
    groups g0=[q0|k2] g1=[q1|v0] g2=[q2|v1] g3=[k0|v2] g4=[k1|-]
    so all q/k land at partition base 0 (k2 realigned via sbuf dma)
    and all v at base 64 (transposed with a base-64 stacked identity).
  - v natural layout via PE transposes + ones column -> AV lhsT [128,65]
  - scoresT chunks: lhsT = kT[64, 128keys], rhs = qT[64, qspan],
    psum regions [128,<=1024], one ACT exp (scale=1/8 folded) per region
    writing bf16 expT; causal corners fixed post-exp (DVE 0/1-mask mult).
  - AV: O[0:65, q] += [v|1].T @ expT (denominator rides along as row 64);
    trunk chunk c=0 spans all queries for both branches so it is the
    unique start=True touch per psum bank (multiple start=True matmuls
    in one 512-col bank wipe each other's accumulation on TRN2).
  - psO is double-buffered, so softmax normalize runs fully off the
    critical path: ACT copies the denom row to sbuf, a DRAM-roundtrip
    DMA broadcasts it to [64,T], DVE reciprocal_approx_fast + mult ->
    bf16 aT.  The proj(1)-gating head (1,2) instead uses a short PE
    ones-matmul broadcast chain in 320-col quarters (tensor queue is
    idle after its last AV), so proj(1) unblocks ~2us later.
  - proj: out[tok,768] = aT(2 k-chunks: 128+64).T @ Wp rows, psum -> sbuf
    -> DRAM partials.  Branch-0 proj is emitted inside branch-1's
    attention stream so the tensor queue never stalls on normalize.
  - startup: input DMA issues are spread over the sync/scalar/gpsimd
    queues (issue cost ~0.65us each, serial per queue) so the first QKV
    matmul starts as soon as wq[0]+xt[0] land (~12us incl preamble).
"""
import sys

sys.path.insert(0, "/opt/trn_rl_repo")

import numpy as np
import ml_dtypes

import concourse.bacc as bacc
import concourse.mybir as mybir
import concourse.tile as tile
from concourse.bass_utils import run_bass_kernel_spmd

F32 = mybir.dt.float32
BF16 = mybir.dt.bfloat16
BF = ml_dtypes.bfloat16

B, NB, BS = 2, 20, 64
D, H = 768, 12
DH = D // H                      # 64
T = NB * BS                      # 1280 tokens per (batch, branch)
DC = D // 128                    # 6 d-chunks
NG = 5                           # qkv col groups of 128
NP = NB // 2                     # 10 key-chunk pairs
H3 = 3                           # heads per core
REG_W = 512                      # scoresT psum region width (1 psum bank)
SCALE = 1.0 / np.sqrt(DH)


# ---------------------------------------------------------------- device IR


def _chunk_plan(e):
    """Ordered (kind, idx, qoff, width) score-chunk segments for branch e.

    kind: 's' self (keys = k_e pair idx), 't' trunk (keys = k0 pair idx).
    Trunk chunk c spans queries [128c, T) for BOTH branches (for e=1 the
    first 64 cols are fully masked) so that trunk c=0 covers every column
    and is the unique start=True touch per psum bank — multiple start=True
    matmuls in one 512-col psum bank wipe each other's accumulation.
    Self segments (e=1) accumulate afterwards with start=False.
    """
    segs = []
    if e == 0:
        for c in range(NP):
            segs.append(("t", c, 128 * c, T - 128 * c))
    else:
        # c=0 spans ALL queries (the unique start=True per psum bank);
        # later chunks start at 128c+64 (block 2c sees neither key block)
        segs.append(("t", 0, 0, T))
        for c in range(1, NP):
            segs.append(("t", c, 128 * c + 64, T - 128 * c - 64))
        for p in range(NP):
            segs.append(("s", p, 128 * p, 128))
    return segs


def build(debug=False):
    nc = bacc.Bacc()

    xt = nc.dram_tensor("xt", [DC, 128, T], BF16, kind="ExternalInput")
    xs = nc.dram_tensor("xs", [DC, 128, T], BF16, kind="ExternalInput")
    wq = nc.dram_tensor("wq", [DC, 128, NG * 128], BF16, kind="ExternalInput")
    bq = nc.dram_tensor("bq", [128, NG], F32, kind="ExternalInput")
    wp = nc.dram_tensor("wp", [2, 128, D], BF16, kind="ExternalInput")
    cst = nc.dram_tensor("cst", [128, 448], BF16, kind="ExternalInput")
    o0 = nc.dram_tensor("o0", [T, D], BF16, kind="ExternalOutput")
    o1 = nc.dram_tensor("o1", [T, D], BF16, kind="ExternalOutput")
    outs = (o0, o1)
    if debug:
        dqv = nc.dram_tensor("dqv", [128, NG * T], BF16, kind="ExternalOutput")
        dat = nc.dram_tensor("dat", [128, T], BF16, kind="ExternalOutput")
        dvna = nc.dram_tensor("dvna", [128, NP * H3 * 65], BF16, kind="ExternalOutput")

    with tile.TileContext(nc) as tc:
        with (
            tc.tile_pool(name="consts", bufs=1) as cp,
            tc.tile_pool(name="big", bufs=1) as bp,
            tc.tile_pool(name="xtp", bufs=2 * DC) as xtp,
            tc.tile_pool(name="expp", bufs=2) as expp,
            tc.tile_pool(name="outst", bufs=3) as outst,
            tc.tile_pool(name="denp", bufs=2) as denp,
            tc.tile_pool(name="denq", bufs=4) as denq,
            tc.tile_pool(name="rbp", bufs=2) as rbp,
            tc.tile_pool(name="rbip", bufs=3) as rbip,
            tc.tile_pool(name="scrp", bufs=8, space="DRAM") as scrp,
            tc.tile_pool(name="work", bufs=2, space="PSUM") as work,
            tc.tile_pool(name="psO", bufs=2, space="PSUM") as psO,
        ):
            # ---- constants & inputs.  DMA issue order matters: the first
            # QKV matmul needs wq[0] + xt[0], so those go first on the sync
            # queue; everything else is spread across scalar/gpsimd queues.
            # spread the startup input DMA issues over four engine queues
            # (issue cost is ~0.65us each, serial per queue)
            wq_sb = [cp.tile([128, NG * 128], BF16, name=f"wq{dc}") for dc in range(DC)]
            xtiles_s = [[], []]
            for dc in range(DC):
                xtile = xtp.tile([128, T], BF16, tag="xt")
                xtiles_s[0].append(xtile)
            for dc in range(DC):
                (nc.sync, nc.gpsimd)[dc % 2].dma_start(
                    xtiles_s[0][dc][:], xt[dc]
                )
                nc.scalar.dma_start(wq_sb[dc][:], wq[dc])
            bq_sb = cp.tile([128, NG], F32)
            nc.scalar.dma_start(bq_sb[:], bq[:])
            wp_sb = cp.tile([128, 2, D], BF16)
            nc.scalar.dma_start(wp_sb[:], wp[:].rearrange("c p f -> p c f"))
            cst_sb = cp.tile([128, 448], BF16)
            nc.scalar.dma_start(cst_sb[:], cst[:])
            i2_sb = cst_sb[:, 0:64]
            m0_sb = cst_sb[:, 64:192]
            m1_sb = cst_sb[:, 192:320]
            ms_sb = cst_sb[:, 320:448]
            for dc in range(DC):
                xtile = xtp.tile([128, T], BF16, tag="xs")
                nc.gpsimd.dma_start(xtile[:], xs[dc])
                xtiles_s[1].append(xtile)

            # ---- persistent per-source tensors
            QV, K2, VNA = [], [], []
            for s in range(2):
                qv = bp.tile([128, NG, T], BF16, name=f"qv{s}")
                k2 = bp.tile([64, T], BF16, name=f"k2{s}")
                vna = bp.tile([128, NP, H3, 65], BF16, name=f"vna{s}")
                nc.gpsimd.memset(vna[:, :, :, 64:65], 1.0)
                QV.append(qv)
                K2.append(k2)
                VNA.append(vna)
            aT01 = [bp.tile([128, T], BF16, name=f"a01_{e}") for e in range(2)]
            aT2 = [bp.tile([64, T], BF16, name=f"a2_{e}") for e in range(2)]

            # views --------------------------------------------------------
            def qT(s, h):
                return QV[s][0:64, h, :]

            def kT(s, h):
                return (QV[s][0:64, 3, :], QV[s][0:64, 4, :], K2[s][:, :])[h]

            def vT(s, h):  # partition base 64
                return QV[s][64:128, 1 + h, :]

            # ---- phase 1: qkv projections -> QV
            for s in range(2):
                xtiles = xtiles_s[s]
                for g in range(NG):
                    for lo, w in ((0, 512), (512, 512), (1024, 256)):
                        pg = work.tile([128, REG_W], F32, tag="work")
                        for dc in range(DC):
                            nc.tensor.matmul(
                                pg[:, 0:w],
                                wq_sb[dc][:, 128 * g : 128 * (g + 1)],
                                xtiles[dc][:, lo : lo + w],
                                start=(dc == 0),
                                stop=(dc == DC - 1),
                            )
                        nc.vector.tensor_scalar_add(
                            QV[s][:, g, lo : lo + w], pg[:, 0:w], bq_sb[:, g : g + 1]
                        )
                # realign k2 (group 0 high half) to partition base 0
                nc.sync.dma_start(K2[s][:], QV[s][64:128, 0, :])

                # ---- phase 2: v natural layout (+ones col already DMA'd)
                for h in range(H3):
                    pt = work.tile([128, 2 * REG_W], BF16, tag="work")
                    for tch in range(NP):
                        nc.tensor.transpose(
                            pt[:, 64 * tch : 64 * (tch + 1)],
                            vT(s, h)[:, 128 * tch : 128 * (tch + 1)],
                            i2_sb[64:128, :],
                        )
                    nc.vector.tensor_copy(
                        VNA[s][:, :, h, 0:64],
                        pt[:, 0 : 64 * NP].rearrange("p (tc d) -> p tc d", d=64),
                    )

            # ---- phase 3+4: attention per (branch, head), proj interleaved
            def attn_head(e, h):
                sq = 0 if e == 0 else 1
                O = psO.tile([128, 1280], F32, tag="O")

                # pack score segments into psum regions
                segs = _chunk_plan(e)
                regions = []  # parts: (kind, idx, qoff, loc, w, choff)
                cur, used = [], 0
                for kind, idx, qoff, width in segs:
                    off = 0
                    while off < width:
                        if REG_W - used < 256:
                            regions.append((cur, used))
                            cur, used = [], 0
                        w = min(width - off, REG_W - used)
                        cur.append((kind, idx, qoff + off, used, w, off))
                        used += w
                        off += w
                if cur:
                    regions.append((cur, used))

                target = (aT01[e][0:64, :], aT01[e][64:128, :], aT2[e][:, :])[h]
                last = e == 1 and h == 2

                def norm_bcast():
                    # off-critical-path: DRAM-roundtrip denominator broadcast
                    # (psO is double-buffered, so holding O is free).  The
                    # DVE recip+mult are DEFERRED one head: emitting them now
                    # would stall the in-order DVE queue (recip waits on the
                    # roundtrip) and the next head's masks behind it.
                    den = denp.tile([1, T], F32, tag="den")
                    nc.scalar.activation(
                        den[:], O[64:65, :], mybir.ActivationFunctionType.Copy
                    )
                    scr = scrp.tile([1, T], F32, tag="scr")
                    nc.gpsimd.dma_start(scr[:], den[:])
                    rb = rbp.tile([64, T], F32, tag="rb")
                    nc.gpsimd.dma_start(rb[:], scr[:].to_broadcast([64, T]))
                    Oc, tgt = O, target

                    def finish():
                        rbi = rbip.tile([64, T], F32, tag="rbi")
                        nc.vector.reciprocal_approx_fast(rbi[:], rb[:])
                        nc.vector.tensor_tensor(
                            tgt, Oc[0:64, :], rbi[:], mybir.AluOpType.mult
                        )

                    return finish

                def norm_pe_batched(qw):
                    # fast path for the proj-gating head: PE ones-matmul
                    # broadcast (tensor queue is idle after the last AV).
                    # Emit phase-major: cross-engine semaphore chains follow
                    # program order, so per-chunk emission serializes.
                    hfs = list(range(0, T, qw))
                    dqs = {}
                    for hf in hfs:
                        dq = denq.tile([1, qw], BF16, tag="dq")
                        nc.scalar.activation(
                            dq[:],
                            O[64:65, hf : hf + qw],
                            mybir.ActivationFunctionType.Copy,
                        )
                        dqs[hf] = dq
                    rbqs = {}
                    for hf in hfs:
                        rbq = work.tile([64, REG_W], F32, tag="work")
                        nc.tensor.matmul(
                            rbq[:, 0:qw],
                            ms_sb[0:1, 0:64],
                            dqs[hf][:],
                            start=True,
                            stop=True,
                        )
                        rbqs[hf] = rbq
                    for hf in hfs:
                        rbi = rbip.tile([64, qw], F32, tag="rbiq")
                        nc.vector.reciprocal_approx_fast(rbi[:], rbqs[hf][:, 0:qw])
                        nc.vector.tensor_tensor(
                            target[:, hf : hf + qw],
                            O[0:64, hf : hf + qw],
                            rbi[:],
                            mybir.AluOpType.mult,
                        )

                def emit_scores(parts, used):
                    rt = work.tile([128, REG_W], F32, tag="work")
                    et = expp.tile([128, REG_W], BF16, tag="expT")
                    for kind, idx, qo, loc, w, choff in parts:
                        kv = kT(sq, h) if kind == "s" else kT(0, h)
                        lhsT = kv[:, 128 * idx : 128 * (idx + 1)]
                        rhs = qT(sq, h)[:, qo : qo + w]
                        # split at psum bank boundaries (512 within rt)
                        p0 = 0
                        while p0 < w:
                            bw = min(w - p0, 512 - ((loc + p0) % 512))
                            nc.tensor.matmul(
                                rt[:, loc + p0 : loc + p0 + bw],
                                lhsT,
                                rhs[:, p0 : p0 + bw],
                                start=True,
                                stop=True,
                            )
                            p0 += bw
                    nc.scalar.activation(
                        et[:, 0:used],
                        rt[:, 0:used],
                        mybir.ActivationFunctionType.Exp,
                        bias=0.0,
                        scale=float(SCALE),
                    )
                    # causal corner fixes (chunk-local cols 0:128 / 0:64)
                    for kind, idx, qo, loc, w, choff in parts:
                        if kind == "s":
                            msk, mw, mo = ms_sb, 128, 0
                        elif e == 0:
                            msk, mw, mo = m0_sb, 128, 0
                        elif idx == 0:
                            msk, mw, mo = m1_sb, 128, 0
                        else:
                            # narrow e=1 trunk chunk: corner is block 2c+1
                            # queries only -> the 64:128 half of m1
                            msk, mw, mo = m1_sb, 64, 64
                        if choff < mw:
                            cw = min(mw - choff, w)
                            nc.vector.tensor_tensor(
                                et[:, loc : loc + cw],
                                et[:, loc : loc + cw],
                                msk[:, mo + choff : mo + choff + cw],
                                mybir.AluOpType.mult,
                            )
                    return et

                def emit_av(parts, et):
                    # AV accumulate into O; trunk c=0 is the unique first
                    # touch (one start=True per psum bank)
                    for kind, idx, qo, loc, w, choff in parts:
                        first = kind == "t" and idx == 0
                        vsrc = VNA[sq] if kind == "s" else VNA[0]
                        lhsT = vsrc[:, idx, h, :]
                        p0 = 0
                        while p0 < w:
                            q0 = qo + p0
                            bw = min(w - p0, 512 - (q0 % 512))
                            nc.tensor.matmul(
                                O[0:65, q0 : q0 + bw],
                                lhsT,
                                et[:, loc + p0 : loc + p0 + bw],
                                start=first,
                                stop=False,
                                skip_group_check=True,
                            )
                            p0 += bw

                # software-pipeline by one region: the tensor queue runs
                # scores r+1 while ACT computes exp r, so AV r never waits
                pend = None
                for parts, used in regions:
                    et = emit_scores(parts, used)
                    if pend is not None:
                        emit_av(*pend)
                    pend = (parts, et)
                emit_av(*pend)
                # flush the previous head's deferred normalize (its
                # broadcast landed while this head was computing)
                while pending_norm:
                    pending_norm.pop(0)()
                if last:
                    norm_pe_batched(320)
                else:
                    pending_norm.append(norm_bcast())

            def proj(e):
                for m in range(NP):
                    ot = outst.tile([128, D], BF16, tag="ot")
                    for lo, w in ((0, 512), (512, 256)):
                        pp = work.tile([128, REG_W], F32, tag="work")
                        nc.tensor.matmul(
                            pp[:, 0:w],
                            aT01[e][:, 128 * m : 128 * (m + 1)],
                            wp_sb[:, 0, lo : lo + w],
                            start=True,
                            stop=False,
                        )
                        nc.tensor.matmul(
                            pp[:, 0:w],
                            aT2[e][:, 128 * m : 128 * (m + 1)],
                            wp_sb[0:64, 1, lo : lo + w],
                            start=False,
                            stop=True,
                        )
                        nc.vector.tensor_copy(ot[:, lo : lo + w], pp[:, 0:w])
                    nc.sync.dma_start(
                        outs[e][128 * m : 128 * (m + 1), :], ot[:]
                    )

            pending_norm = []
            attn_head(0, 0)
            attn_head(0, 1)
            attn_head(0, 2)
            attn_head(1, 0)
            proj(0)
            attn_head(1, 1)
            attn_head(1, 2)
            proj(1)

    nc.finalize()
    return nc


# ---------------------------------------------------------------- host side

_NC = None


def _get_nc():
    global _NC
    if _NC is None:
        _NC = build()
    return _NC


def _consts():
    i2 = np.zeros((128, 64), np.float32)
    i2[:64] = np.eye(64, dtype=np.float32)
    i2[64:] = np.eye(64, dtype=np.float32)
    p = np.arange(128)[:, None]
    x = np.arange(128)[None, :]
    m0 = (p <= x).astype(np.float32)
    ms = np.where(x < 64, p <= x, (p >= 64) & (p <= x)).astype(np.float32)
    m1 = np.zeros((128, 128), np.float32)
    m1[0:64, 64:128] = 1.0
    cst = np.concatenate([i2, m0, m1, ms], axis=1)
    return dict(cst=cst.astype(BF))


def _core_inputs(x0, x1, w_attn, b_attn, w_proj, consts):
    """Build the 8 per-core input maps. Core order: (b, G) row-major."""
    maps = []
    xT = [
        [x[b].reshape(T, D).T.astype(BF).reshape(DC, 128, T) for b in range(B)]
        for x in (x0, x1)
    ]
    for b in range(B):
        for G in range(4):
            gh = [3 * G + h for h in range(H3)]
            qc = [768 + g * 64 + np.arange(64) for g in gh]
            kc = [1536 + g * 64 + np.arange(64) for g in gh]
            vc = [0 + g * 64 + np.arange(64) for g in gh]
            groups = [
                np.concatenate([qc[0], kc[2]]),
                np.concatenate([qc[1], vc[0]]),
                np.concatenate([qc[2], vc[1]]),
                np.concatenate([kc[0], vc[2]]),
                np.concatenate([kc[1], kc[1]]),  # pad half unused
            ]
            cols = np.concatenate(groups)
            wqm = w_attn[:, cols].copy()
            wqm[:, 4 * 128 + 64 :] = 0.0
            bqm = b_attn[cols].reshape(NG, 128).T.copy()
            bqm[64:, 4] = 0.0
            wpm = np.zeros((2, 128, D), np.float32)
            wpm[0] = w_proj[3 * G * 64 : 3 * G * 64 + 128]
            wpm[1, 0:64] = w_proj[3 * G * 64 + 128 : 3 * G * 64 + 192]
            maps.append(
                dict(
                    xt=xT[0][b],
                    xs=xT[1][b],
                    wq=wqm.astype(BF).reshape(DC, 128, NG * 128),
                    bq=np.ascontiguousarray(bqm, np.float32),
                    wp=wpm.astype(BF),
                    **consts,
                )
            )
    return maps


def kernel(x0, x1, w_attn, b_attn, w_proj, b_proj, _trace=False):
    x0 = np.asarray(x0, np.float32)
    x1 = np.asarray(x1, np.float32)
    w_attn = np.asarray(w_attn, np.float32)
    b_attn = np.asarray(b_attn, np.float32)
    w_proj = np.asarray(w_proj, np.float32)
    b_proj = np.asarray(b_proj, np.float32)

    nc = _get_nc()
    maps = _core_inputs(x0, x1, w_attn, b_attn, w_proj, _consts())
    if _trace:
        res = run_bass_kernel_spmd(
            nc, maps, core_ids=list(range(8)), trace=True
        )
    else:
        # an ambient BASS_TRACE=1 would route run_bass_kernel_spmd into the
        # NTFF path, which crashes on this image (antenv.axon_hooks is
        # missing) — pin the non-trace path for the plain call
        import os

        prev = os.environ.get("BASS_NEVER_TRACE")
        os.environ["BASS_NEVER_TRACE"] = "1"
        try:
            res = run_bass_kernel_spmd(
                nc, maps, core_ids=list(range(8)), trace=False
            )
        finally:
            if prev is None:
                os.environ.pop("BASS_NEVER_TRACE", None)
            else:
                os.environ["BASS_NEVER_TRACE"] = prev

    out = [np.zeros((B, T, D), np.float32) for _ in range(2)]
    for ci, r in enumerate(res.results):
        b = ci // 4
        out[0][b] += np.asarray(r["o0"], np.float32)
        out[1][b] += np.asarray(r["o1"], np.float32)
    out0 = (out[0] + b_proj).reshape(B, NB, BS, D)
    out1 = (out[1] + b_proj).reshape(B, NB, BS, D)
    if _trace:
        kernel._last = res
    return out0, out1


if __name__ == "__main__":
    rng = np.random.default_rng(0)
    x0 = rng.standard_normal((B, NB, BS, D), dtype=np.float32)
    x1 = rng.standard_normal((B, NB, BS, D), dtype=np.float32)
    wa = rng.standard_normal((D, 3 * D), dtype=np.float32) * 0.02
    ba = np.zeros(3 * D, np.float32)
    wpj = rng.standard_normal((D, D), dtype=np.float32) * 0.02
    bp_ = np.zeros(D, np.float32)
    o0, o1 = kernel(x0, x1, wa, ba, wpj, bp_)
    print("ran", o0.shape, o1.shape, float(np.abs(o0).mean()))


# revision 51
# speedup vs baseline: 1.0251x; 1.0251x over previous
"""BranchingAttention (ViewFormer) Trainium2 Bass kernel.

Problem: two token streams x0 (trunk) / x1, fused qkv projection
(w_attn packs v|q|k), block-causal multi-end attention:
  query token t in block i of branch e attends ALL tokens of trunk
  blocks j < i plus causally (u <= t) its own branch's block i,
joint softmax, out projection.  Returns (out0, out1).

Sharding (8 cores): data-parallel over batch (B=2) x tensor-parallel
over 4 head-groups of 3 heads.  Each core computes BOTH branches for
its 3 heads and emits partial projections; the host sums the 4 head
-group partials per (branch, batch) and adds b_proj.

Per-core device kernel (uniform SPMD program, bf16 matmuls):
  - inputs are host-pre-transposed (xT: [d, tok]) and head-sliced
  - qkv: psum[grp] = sum_dchunk Wg[dc].T @ xT[dc] -> QV sbuf (bf16)
    groups g0=[q0|k2] g1=[q1|v0] g2=[q2|v1] g3=[k0|v2] g4=[k1|-]
    so all q/k land at partition base 0 (k2 realigned via sbuf dma)
    and all v at base 64 (transposed with a base-64 stacked identity).
  - v natural layout via PE transposes + ones column -> AV lhsT [128,65]
  - scoresT chunks: lhsT = kT[64, 128keys], rhs = qT[64, qspan],
    psum regions [128,<=1024], one ACT exp (scale=1/8 folded) per region
    writing bf16 expT; causal corners fixed post-exp (DVE 0/1-mask mult).
  - AV: O[0:65, q] += [v|1].T @ expT (denominator rides along as row 64);
    trunk chunk c=0 spans all queries for both branches so it is the
    unique start=True touch per psum bank (multiple start=True matmuls
    in one 512-col bank wipe each other's accumulation on TRN2).
  - psO is double-buffered, so softmax normalize runs fully off the
    critical path: ACT copies the denom row to sbuf, a DRAM-roundtrip
    DMA broadcasts it to [64,T], DVE reciprocal_approx_fast + mult ->
    bf16 aT.  The proj(1)-gating head (1,2) instead uses a short PE
    ones-matmul broadcast chain in 320-col quarters (tensor queue is
    idle after its last AV), so proj(1) unblocks ~2us later.
  - proj: out[tok,768] = aT(2 k-chunks: 128+64).T @ Wp rows, psum -> sbuf
    -> DRAM partials.  Branch-0 proj is emitted inside branch-1's
    attention stream so the tensor queue never stalls on normalize.
  - startup: input DMA issues are spread over the sync/scalar/gpsimd
    queues (issue cost ~0.65us each, serial per queue) so the first QKV
    matmul starts as soon as wq[0]+xt[0] land (~12us incl preamble).
"""
import sys

sys.path.insert(0, "/opt/trn_rl_repo")

import numpy as np
import ml_dtypes

import concourse.bacc as bacc
import concourse.mybir as mybir
import concourse.tile as tile
from concourse.bass_utils import run_bass_kernel_spmd

F32 = mybir.dt.float32
BF16 = mybir.dt.bfloat16
BF = ml_dtypes.bfloat16

B, NB, BS = 2, 20, 64
D, H = 768, 12
DH = D // H                      # 64
T = NB * BS                      # 1280 tokens per (batch, branch)
DC = D // 128                    # 6 d-chunks
NG = 5                           # qkv col groups of 128
NP = NB // 2                     # 10 key-chunk pairs
H3 = 3                           # heads per core
REG_W = 512                      # scoresT psum region width (1 psum bank)
SCALE = 1.0 / np.sqrt(DH)


# ---------------------------------------------------------------- device IR


def _chunk_plan(e):
    """Ordered (kind, idx, qoff, width) score-chunk segments for branch e.

    kind: 's' self (keys = k_e pair idx), 't' trunk (keys = k0 pair idx).
    Trunk chunk c spans queries [128c, T) for BOTH branches (for e=1 the
    first 64 cols are fully masked) so that trunk c=0 covers every column
    and is the unique start=True touch per psum bank — multiple start=True
    matmuls in one 512-col psum bank wipe each other's accumulation.
    Self segments (e=1) accumulate afterwards with start=False.
    """
    segs = []
    if e == 0:
        for c in range(NP):
            segs.append(("t", c, 128 * c, T - 128 * c))
    else:
        # c=0 spans ALL queries (the unique start=True per psum bank);
        # later chunks start at 128c+64 (block 2c sees neither key block)
        segs.append(("t", 0, 0, T))
        for c in range(1, NP):
            segs.append(("t", c, 128 * c + 64, T - 128 * c - 64))
        for p in range(NP):
            segs.append(("s", p, 128 * p, 128))
    return segs


def build(debug=False):
    nc = bacc.Bacc()

    xt = nc.dram_tensor("xt", [DC, 128, T], BF16, kind="ExternalInput")
    xs = nc.dram_tensor("xs", [DC, 128, T], BF16, kind="ExternalInput")
    wq = nc.dram_tensor("wq", [DC, 128, NG * 128], BF16, kind="ExternalInput")
    bq = nc.dram_tensor("bq", [128, NG], F32, kind="ExternalInput")
    wp = nc.dram_tensor("wp", [2, 128, D], BF16, kind="ExternalInput")
    cst = nc.dram_tensor("cst", [128, 448], BF16, kind="ExternalInput")
    o0 = nc.dram_tensor("o0", [T, D], BF16, kind="ExternalOutput")
    o1 = nc.dram_tensor("o1", [T, D], BF16, kind="ExternalOutput")
    outs = (o0, o1)
    if debug:
        dqv = nc.dram_tensor("dqv", [128, NG * T], BF16, kind="ExternalOutput")
        dat = nc.dram_tensor("dat", [128, T], BF16, kind="ExternalOutput")
        dvna = nc.dram_tensor("dvna", [128, NP * H3 * 65], BF16, kind="ExternalOutput")

    with tile.TileContext(nc) as tc:
        with (
            tc.tile_pool(name="consts", bufs=1) as cp,
            tc.tile_pool(name="big", bufs=1) as bp,
            tc.tile_pool(name="xtp", bufs=6 * DC) as xtp,
            tc.tile_pool(name="expp", bufs=2) as expp,
            tc.tile_pool(name="outst", bufs=3) as outst,
            tc.tile_pool(name="denp", bufs=2) as denp,
            tc.tile_pool(name="denq", bufs=4) as denq,
            tc.tile_pool(name="rbp", bufs=2) as rbp,
            tc.tile_pool(name="rbip", bufs=3) as rbip,
            tc.tile_pool(name="scrp", bufs=8, space="DRAM") as scrp,
            tc.tile_pool(name="work", bufs=2, space="PSUM") as work,
            tc.tile_pool(name="psO", bufs=2, space="PSUM") as psO,
        ):
            # ---- constants & inputs.  DMA issue order matters: the first
            # QKV matmul needs wq[0] + xt[0], so those go first on the sync
            # queue; everything else is spread across scalar/gpsimd queues.
            # spread the startup input DMA issues over four engine queues
            # (issue cost is ~0.65us each, serial per queue)
            wq_sb = [cp.tile([128, NG * 128], BF16, name=f"wq{dc}") for dc in range(DC)]
            LOS = ((0, 512), (512, 512), (1024, 256))
            xtiles_s = [[], []]
            for dc in range(DC):
                xts = []
                for li in range(3):
                    xtile = xtp.tile([128, 512], BF16, tag="xt")
                    xts.append(xtile)
                xtiles_s[0].append(xts)
            # issue lo-major so the first QKV regions start after ~1/3 of x
            for dc in range(DC):
                nc.scalar.dma_start(wq_sb[dc][:], wq[dc])
            for li, (lo, w) in enumerate(LOS):
                for dc in range(DC):
                    (nc.sync, nc.gpsimd)[dc % 2].dma_start(
                        xtiles_s[0][dc][li][:, 0:w], xt[dc][:, lo : lo + w]
                    )
            bq_sb = cp.tile([128, NG], F32)
            nc.scalar.dma_start(bq_sb[:], bq[:])
            wp_sb = cp.tile([128, 2, D], BF16)
            nc.scalar.dma_start(wp_sb[:], wp[:].rearrange("c p f -> p c f"))
            cst_sb = cp.tile([128, 448], BF16)
            nc.scalar.dma_start(cst_sb[:], cst[:])
            i2_sb = cst_sb[:, 0:64]
            m0_sb = cst_sb[:, 64:192]
            m1_sb = cst_sb[:, 192:320]
            ms_sb = cst_sb[:, 320:448]
            for dc in range(DC):
                xts = []
                for li, (lo, w) in enumerate(LOS):
                    xtile = xtp.tile([128, 512], BF16, tag="xs")
                    nc.gpsimd.dma_start(xtile[:, 0:w], xs[dc][:, lo : lo + w])
                    xts.append(xtile)
                xtiles_s[1].append(xts)

            # ---- persistent per-source tensors
            QV, K2, VNA = [], [], []
            for s in range(2):
                qv = bp.tile([128, NG, T], BF16, name=f"qv{s}")
                k2 = bp.tile([64, T], BF16, name=f"k2{s}")
                vna = bp.tile([128, NP, H3, 65], BF16, name=f"vna{s}")
                nc.gpsimd.memset(vna[:, :, :, 64:65], 1.0)
                QV.append(qv)
                K2.append(k2)
                VNA.append(vna)
            aT01 = [bp.tile([128, T], BF16, name=f"a01_{e}") for e in range(2)]
            aT2 = [bp.tile([64, T], BF16, name=f"a2_{e}") for e in range(2)]

            # views --------------------------------------------------------
            def qT(s, h):
                return QV[s][0:64, h, :]

            def kT(s, h):
                return (QV[s][0:64, 3, :], QV[s][0:64, 4, :], K2[s][:, :])[h]

            def vT(s, h):  # partition base 64
                return QV[s][64:128, 1 + h, :]

            # ---- phase 1: qkv projections -> QV
            for s in range(2):
                xtiles = xtiles_s[s]
                for li, (lo, w) in enumerate(LOS):
                    for g in range(NG):
                        pg = work.tile([128, REG_W], F32, tag="work")
                        for dc in range(DC):
                            nc.tensor.matmul(
                                pg[:, 0:w],
                                wq_sb[dc][:, 128 * g : 128 * (g + 1)],
                                xtiles[dc][li][:, 0:w],
                                start=(dc == 0),
                                stop=(dc == DC - 1),
                            )
                        nc.vector.tensor_scalar_add(
                            QV[s][:, g, lo : lo + w], pg[:, 0:w], bq_sb[:, g : g + 1]
                        )
                # realign k2 (group 0 high half) to partition base 0
                nc.sync.dma_start(K2[s][:], QV[s][64:128, 0, :])

                # ---- phase 2: v natural layout (+ones col already DMA'd)
                for h in range(H3):
                    pt = work.tile([128, 2 * REG_W], BF16, tag="work")
                    for tch in range(NP):
                        nc.tensor.transpose(
                            pt[:, 64 * tch : 64 * (tch + 1)],
                            vT(s, h)[:, 128 * tch : 128 * (tch + 1)],
                            i2_sb[64:128, :],
                        )
                    nc.vector.tensor_copy(
                        VNA[s][:, :, h, 0:64],
                        pt[:, 0 : 64 * NP].rearrange("p (tc d) -> p tc d", d=64),
                    )

            # ---- phase 3+4: attention per (branch, head), proj interleaved
            def attn_head(e, h):
                sq = 0 if e == 0 else 1
                O = psO.tile([128, 1280], F32, tag="O")

                # pack score segments into psum regions
                segs = _chunk_plan(e)
                regions = []  # parts: (kind, idx, qoff, loc, w, choff)
                cur, used = [], 0
                for kind, idx, qoff, width in segs:
                    off = 0
                    while off < width:
                        if REG_W - used < 256:
                            regions.append((cur, used))
                            cur, used = [], 0
                        w = min(width - off, REG_W - used)
                        cur.append((kind, idx, qoff + off, used, w, off))
                        used += w
                        off += w
                if cur:
                    regions.append((cur, used))

                target = (aT01[e][0:64, :], aT01[e][64:128, :], aT2[e][:, :])[h]
                last = e == 1 and h == 2

                def norm_bcast():
                    # off-critical-path: DRAM-roundtrip denominator broadcast
                    # (psO is double-buffered, so holding O is free).  The
                    # DVE recip+mult are DEFERRED one head: emitting them now
                    # would stall the in-order DVE queue (recip waits on the
                    # roundtrip) and the next head's masks behind it.
                    den = denp.tile([1, T], F32, tag="den")
                    nc.scalar.activation(
                        den[:], O[64:65, :], mybir.ActivationFunctionType.Copy
                    )
                    scr = scrp.tile([1, T], F32, tag="scr")
                    nc.gpsimd.dma_start(scr[:], den[:])
                    rb = rbp.tile([64, T], F32, tag="rb")
                    nc.gpsimd.dma_start(rb[:], scr[:].to_broadcast([64, T]))
                    Oc, tgt = O, target

                    def finish():
                        rbi = rbip.tile([64, T], F32, tag="rbi")
                        nc.vector.reciprocal_approx_fast(rbi[:], rb[:])
                        nc.vector.tensor_tensor(
                            tgt, Oc[0:64, :], rbi[:], mybir.AluOpType.mult
                        )

                    return finish

                def norm_pe_batched(qw):
                    # fast path for the proj-gating head: PE ones-matmul
                    # broadcast (tensor queue is idle after the last AV).
                    # Emit phase-major: cross-engine semaphore chains follow
                    # program order, so per-chunk emission serializes.
                    hfs = list(range(0, T, qw))
                    dqs = {}
                    for hf in hfs:
                        dq = denq.tile([1, qw], BF16, tag="dq")
                        nc.scalar.activation(
                            dq[:],
                            O[64:65, hf : hf + qw],
                            mybir.ActivationFunctionType.Copy,
                        )
                        dqs[hf] = dq
                    rbqs = {}
                    for hf in hfs:
                        rbq = work.tile([64, REG_W], F32, tag="work")
                        nc.tensor.matmul(
                            rbq[:, 0:qw],
                            ms_sb[0:1, 0:64],
                            dqs[hf][:],
                            start=True,
                            stop=True,
                        )
                        rbqs[hf] = rbq
                    for hf in hfs:
                        rbi = rbip.tile([64, qw], F32, tag="rbiq")
                        nc.vector.reciprocal_approx_fast(rbi[:], rbqs[hf][:, 0:qw])
                        nc.vector.tensor_tensor(
                            target[:, hf : hf + qw],
                            O[0:64, hf : hf + qw],
                            rbi[:],
                            mybir.AluOpType.mult,
                        )

                def emit_scores(parts, used):
                    rt = work.tile([128, REG_W], F32, tag="work")
                    et = expp.tile([128, REG_W], BF16, tag="expT")
                    for kind, idx, qo, loc, w, choff in parts:
                        kv = kT(sq, h) if kind == "s" else kT(0, h)
                        lhsT = kv[:, 128 * idx : 128 * (idx + 1)]
                        rhs = qT(sq, h)[:, qo : qo + w]
                        # split at psum bank boundaries (512 within rt)
                        p0 = 0
                        while p0 < w:
                            bw = min(w - p0, 512 - ((loc + p0) % 512))
                            nc.tensor.matmul(
                                rt[:, loc + p0 : loc + p0 + bw],
                                lhsT,
                                rhs[:, p0 : p0 + bw],
                                start=True,
                                stop=True,
                            )
                            p0 += bw
                    nc.scalar.activation(
                        et[:, 0:used],
                        rt[:, 0:used],
                        mybir.ActivationFunctionType.Exp,
                        bias=0.0,
                        scale=float(SCALE),
                    )
                    # causal corner fixes (chunk-local cols 0:128 / 0:64)
                    for kind, idx, qo, loc, w, choff in parts:
                        if kind == "s":
                            msk, mw, mo = ms_sb, 128, 0
                        elif e == 0:
                            msk, mw, mo = m0_sb, 128, 0
                        elif idx == 0:
                            msk, mw, mo = m1_sb, 128, 0
                        else:
                            # narrow e=1 trunk chunk: corner is block 2c+1
                            # queries only -> the 64:128 half of m1
                            msk, mw, mo = m1_sb, 64, 64
                        if choff < mw:
                            cw = min(mw - choff, w)
                            nc.vector.tensor_tensor(
                                et[:, loc : loc + cw],
                                et[:, loc : loc + cw],
                                msk[:, mo + choff : mo + choff + cw],
                                mybir.AluOpType.mult,
                            )
                    return et

                def emit_av(parts, et):
                    # AV accumulate into O; trunk c=0 is the unique first
                    # touch (one start=True per psum bank)
                    for kind, idx, qo, loc, w, choff in parts:
                        first = kind == "t" and idx == 0
                        vsrc = VNA[sq] if kind == "s" else VNA[0]
                        lhsT = vsrc[:, idx, h, :]
                        p0 = 0
                        while p0 < w:
                            q0 = qo + p0
                            bw = min(w - p0, 512 - (q0 % 512))
                            nc.tensor.matmul(
                                O[0:65, q0 : q0 + bw],
                                lhsT,
                                et[:, loc + p0 : loc + p0 + bw],
                                start=first,
                                stop=False,
                                skip_group_check=True,
                            )
                            p0 += bw

                # software-pipeline by one region: the tensor queue runs
                # scores r+1 while ACT computes exp r, so AV r never waits
                pend = None
                for parts, used in regions:
                    et = emit_scores(parts, used)
                    if pend is not None:
                        emit_av(*pend)
                    pend = (parts, et)
                emit_av(*pend)
                # flush the previous head's deferred normalize (its
                # broadcast landed while this head was computing)
                while pending_norm:
                    pending_norm.pop(0)()
                if last:
                    norm_pe_batched(320)
                else:
                    pending_norm.append(norm_bcast())

            def proj(e):
                for m in range(NP):
                    ot = outst.tile([128, D], BF16, tag="ot")
                    for lo, w in ((0, 512), (512, 256)):
                        pp = work.tile([128, REG_W], F32, tag="work")
                        nc.tensor.matmul(
                            pp[:, 0:w],
                            aT01[e][:, 128 * m : 128 * (m + 1)],
                            wp_sb[:, 0, lo : lo + w],
                            start=True,
                            stop=False,
                        )
                        nc.tensor.matmul(
                            pp[:, 0:w],
                            aT2[e][:, 128 * m : 128 * (m + 1)],
                            wp_sb[0:64, 1, lo : lo + w],
                            start=False,
                            stop=True,
                        )
                        nc.vector.tensor_copy(ot[:, lo : lo + w], pp[:, 0:w])
                    nc.sync.dma_start(
                        outs[e][128 * m : 128 * (m + 1), :], ot[:]
                    )

            pending_norm = []
            attn_head(0, 0)
            attn_head(0, 1)
            attn_head(0, 2)
            attn_head(1, 0)
            proj(0)
            attn_head(1, 1)
            attn_head(1, 2)
            proj(1)

    nc.finalize()
    return nc


# ---------------------------------------------------------------- host side

_NC = None


def _get_nc():
    global _NC
    if _NC is None:
        _NC = build()
    return _NC


def _consts():
    i2 = np.zeros((128, 64), np.float32)
    i2[:64] = np.eye(64, dtype=np.float32)
    i2[64:] = np.eye(64, dtype=np.float32)
    p = np.arange(128)[:, None]
    x = np.arange(128)[None, :]
    m0 = (p <= x).astype(np.float32)
    ms = np.where(x < 64, p <= x, (p >= 64) & (p <= x)).astype(np.float32)
    m1 = np.zeros((128, 128), np.float32)
    m1[0:64, 64:128] = 1.0
    cst = np.concatenate([i2, m0, m1, ms], axis=1)
    return dict(cst=cst.astype(BF))


def _core_inputs(x0, x1, w_attn, b_attn, w_proj, consts):
    """Build the 8 per-core input maps. Core order: (b, G) row-major."""
    maps = []
    xT = [
        [x[b].reshape(T, D).T.astype(BF).reshape(DC, 128, T) for b in range(B)]
        for x in (x0, x1)
    ]
    for b in range(B):
        for G in range(4):
            gh = [3 * G + h for h in range(H3)]
            qc = [768 + g * 64 + np.arange(64) for g in gh]
            kc = [1536 + g * 64 + np.arange(64) for g in gh]
            vc = [0 + g * 64 + np.arange(64) for g in gh]
            groups = [
                np.concatenate([qc[0], kc[2]]),
                np.concatenate([qc[1], vc[0]]),
                np.concatenate([qc[2], vc[1]]),
                np.concatenate([kc[0], vc[2]]),
                np.concatenate([kc[1], kc[1]]),  # pad half unused
            ]
            cols = np.concatenate(groups)
            wqm = w_attn[:, cols].copy()
            wqm[:, 4 * 128 + 64 :] = 0.0
            bqm = b_attn[cols].reshape(NG, 128).T.copy()
            bqm[64:, 4] = 0.0
            wpm = np.zeros((2, 128, D), np.float32)
            wpm[0] = w_proj[3 * G * 64 : 3 * G * 64 + 128]
            wpm[1, 0:64] = w_proj[3 * G * 64 + 128 : 3 * G * 64 + 192]
            maps.append(
                dict(
                    xt=xT[0][b],
                    xs=xT[1][b],
                    wq=wqm.astype(BF).reshape(DC, 128, NG * 128),
                    bq=np.ascontiguousarray(bqm, np.float32),
                    wp=wpm.astype(BF),
                    **consts,
                )
            )
    return maps


def kernel(x0, x1, w_attn, b_attn, w_proj, b_proj, _trace=False):
    x0 = np.asarray(x0, np.float32)
    x1 = np.asarray(x1, np.float32)
    w_attn = np.asarray(w_attn, np.float32)
    b_attn = np.asarray(b_attn, np.float32)
    w_proj = np.asarray(w_proj, np.float32)
    b_proj = np.asarray(b_proj, np.float32)

    nc = _get_nc()
    maps = _core_inputs(x0, x1, w_attn, b_attn, w_proj, _consts())
    if _trace:
        res = run_bass_kernel_spmd(
            nc, maps, core_ids=list(range(8)), trace=True
        )
    else:
        # an ambient BASS_TRACE=1 would route run_bass_kernel_spmd into the
        # NTFF path, which crashes on this image (antenv.axon_hooks is
        # missing) — pin the non-trace path for the plain call
        import os

        prev = os.environ.get("BASS_NEVER_TRACE")
        os.environ["BASS_NEVER_TRACE"] = "1"
        try:
            res = run_bass_kernel_spmd(
                nc, maps, core_ids=list(range(8)), trace=False
            )
        finally:
            if prev is None:
                os.environ.pop("BASS_NEVER_TRACE", None)
            else:
                os.environ["BASS_NEVER_TRACE"] = prev

    out = [np.zeros((B, T, D), np.float32) for _ in range(2)]
    for ci, r in enumerate(res.results):
        b = ci // 4
        out[0][b] += np.asarray(r["o0"], np.float32)
        out[1][b] += np.asarray(r["o1"], np.float32)
    out0 = (out[0] + b_proj).reshape(B, NB, BS, D)
    out1 = (out[1] + b_proj).reshape(B, NB, BS, D)
    if _trace:
        kernel._last = res
    return out0, out1


if __name__ == "__main__":
    rng = np.random.default_rng(0)
    x0 = rng.standard_normal((B, NB, BS, D), dtype=np.float32)
    x1 = rng.standard_normal((B, NB, BS, D), dtype=np.float32)
    wa = rng.standard_normal((D, 3 * D), dtype=np.float32) * 0.02
    ba = np.zeros(3 * D, np.float32)
    wpj = rng.standard_normal((D, D), dtype=np.float32) * 0.02
    bp_ = np.zeros(D, np.float32)
    o0, o1 = kernel(x0, x1, wa, ba, wpj, bp_)
    print("ran", o0.shape, o1.shape, float(np.abs(o0).mean()))


# revision 52
# speedup vs baseline: 1.0337x; 1.0084x over previous
"""BranchingAttention (ViewFormer) Trainium2 Bass kernel.

Problem: two token streams x0 (trunk) / x1, fused qkv projection
(w_attn packs v|q|k), block-causal multi-end attention:
  query token t in block i of branch e attends ALL tokens of trunk
  blocks j < i plus causally (u <= t) its own branch's block i,
joint softmax, out projection.  Returns (out0, out1).

Sharding (8 cores): data-parallel over batch (B=2) x tensor-parallel
over 4 head-groups of 3 heads.  Each core computes BOTH branches for
its 3 heads and emits partial projections; the host sums the 4 head
-group partials per (branch, batch) and adds b_proj.

Per-core device kernel (uniform SPMD program, bf16 matmuls):
  - inputs are host-pre-transposed (xT: [d, tok]) and head-sliced
  - qkv: psum[grp] = sum_dchunk Wg[dc].T @ xT[dc] -> QV sbuf (bf16)
    groups g0=[q0|k2] g1=[q1|v0] g2=[q2|v1] g3=[k0|v2] g4=[k1|-]
    so all q/k land at partition base 0 (k2 realigned via sbuf dma)
    and all v at base 64 (transposed with a base-64 stacked identity).
  - v natural layout via PE transposes + ones column -> AV lhsT [128,65]
  - scoresT chunks: lhsT = kT[64, 128keys], rhs = qT[64, qspan],
    psum regions [128,<=1024], one ACT exp (scale=1/8 folded) per region
    writing bf16 expT; causal corners fixed post-exp (DVE 0/1-mask mult).
  - AV: O[0:65, q] += [v|1].T @ expT (denominator rides along as row 64);
    trunk chunk c=0 spans all queries for both branches so it is the
    unique start=True touch per psum bank (multiple start=True matmuls
    in one 512-col bank wipe each other's accumulation on TRN2).
  - psO is double-buffered, so softmax normalize runs fully off the
    critical path: ACT copies the denom row to sbuf, a DRAM-roundtrip
    DMA broadcasts it to [64,T], DVE reciprocal_approx_fast + mult ->
    bf16 aT.  The proj(1)-gating head (1,2) instead uses a short PE
    ones-matmul broadcast chain in 320-col quarters (tensor queue is
    idle after its last AV), so proj(1) unblocks ~2us later.
  - proj: out[tok,768] = aT(2 k-chunks: 128+64).T @ Wp rows, psum -> sbuf
    -> DRAM partials.  Branch-0 proj is emitted inside branch-1's
    attention stream so the tensor queue never stalls on normalize.
  - startup: input DMA issues are spread over the sync/scalar/gpsimd
    queues (issue cost ~0.65us each, serial per queue) so the first QKV
    matmul starts as soon as wq[0]+xt[0] land (~12us incl preamble).
"""
import sys

sys.path.insert(0, "/opt/trn_rl_repo")

import numpy as np
import ml_dtypes

import concourse.bacc as bacc
import concourse.mybir as mybir
import concourse.tile as tile
from concourse.bass_utils import run_bass_kernel_spmd

F32 = mybir.dt.float32
BF16 = mybir.dt.bfloat16
BF = ml_dtypes.bfloat16

B, NB, BS = 2, 20, 64
D, H = 768, 12
DH = D // H                      # 64
T = NB * BS                      # 1280 tokens per (batch, branch)
DC = D // 128                    # 6 d-chunks
NG = 5                           # qkv col groups of 128
NP = NB // 2                     # 10 key-chunk pairs
H3 = 3                           # heads per core
REG_W = 512                      # scoresT psum region width (1 psum bank)
SCALE = 1.0 / np.sqrt(DH)


# ---------------------------------------------------------------- device IR


def _chunk_plan(e):
    """Ordered (kind, idx, qoff, width) score-chunk segments for branch e.

    kind: 's' self (keys = k_e pair idx), 't' trunk (keys = k0 pair idx).
    Trunk chunk c spans queries [128c, T) for BOTH branches (for e=1 the
    first 64 cols are fully masked) so that trunk c=0 covers every column
    and is the unique start=True touch per psum bank — multiple start=True
    matmuls in one 512-col psum bank wipe each other's accumulation.
    Self segments (e=1) accumulate afterwards with start=False.
    """
    segs = []
    if e == 0:
        for c in range(NP):
            segs.append(("t", c, 128 * c, T - 128 * c))
    else:
        # c=0 spans ALL queries (the unique start=True per psum bank);
        # later chunks start at 128c+64 (block 2c sees neither key block)
        segs.append(("t", 0, 0, T))
        for c in range(1, NP):
            segs.append(("t", c, 128 * c + 64, T - 128 * c - 64))
        for p in range(NP):
            segs.append(("s", p, 128 * p, 128))
    return segs


def build(debug=False):
    nc = bacc.Bacc()

    xt = nc.dram_tensor("xt", [DC, 128, T], BF16, kind="ExternalInput")
    xs = nc.dram_tensor("xs", [DC, 128, T], BF16, kind="ExternalInput")
    wq = nc.dram_tensor("wq", [DC, 128, NG * 128], BF16, kind="ExternalInput")
    bq = nc.dram_tensor("bq", [128, NG], F32, kind="ExternalInput")
    wp = nc.dram_tensor("wp", [2, 128, D], BF16, kind="ExternalInput")
    cst = nc.dram_tensor("cst", [128, 448], BF16, kind="ExternalInput")
    o0 = nc.dram_tensor("o0", [T, D], BF16, kind="ExternalOutput")
    o1 = nc.dram_tensor("o1", [T, D], BF16, kind="ExternalOutput")
    outs = (o0, o1)
    if debug:
        dqv = nc.dram_tensor("dqv", [128, NG * T], BF16, kind="ExternalOutput")
        dat = nc.dram_tensor("dat", [128, T], BF16, kind="ExternalOutput")
        dvna = nc.dram_tensor("dvna", [128, NP * H3 * 65], BF16, kind="ExternalOutput")

    with tile.TileContext(nc) as tc:
        with (
            tc.tile_pool(name="consts", bufs=1) as cp,
            tc.tile_pool(name="big", bufs=1) as bp,
            tc.tile_pool(name="xtp", bufs=6 * DC) as xtp,
            tc.tile_pool(name="expp", bufs=2) as expp,
            tc.tile_pool(name="outst", bufs=3) as outst,
            tc.tile_pool(name="denp", bufs=2) as denp,
            tc.tile_pool(name="denq", bufs=4) as denq,
            tc.tile_pool(name="rbp", bufs=2) as rbp,
            tc.tile_pool(name="rbip", bufs=3) as rbip,
            tc.tile_pool(name="scrp", bufs=8, space="DRAM") as scrp,
            tc.tile_pool(name="work", bufs=2, space="PSUM") as work,
            tc.tile_pool(name="psO", bufs=2, space="PSUM") as psO,
        ):
            # ---- constants & inputs.  DMA issue order matters: the first
            # QKV matmul needs wq[0] + xt[0], so those go first on the sync
            # queue; everything else is spread across scalar/gpsimd queues.
            # spread the startup input DMA issues over four engine queues
            # (issue cost is ~0.65us each, serial per queue)
            wq_sb = [cp.tile([128, NG * 128], BF16, name=f"wq{dc}") for dc in range(DC)]
            LOS = ((0, 512), (512, 512), (1024, 256))
            xtiles_s = [[], []]
            for dc in range(DC):
                xts = []
                for li in range(3):
                    xtile = xtp.tile([128, 512], BF16, tag="xt")
                    xts.append(xtile)
                xtiles_s[0].append(xts)
            # issue lo-major, with the 12 first-needed transfers (wq +
            # xt lo=0) balanced across all three queues' engine pools
            Q3 = (nc.sync, nc.scalar, nc.gpsimd)
            for dc in range(DC):
                Q3[dc % 3].dma_start(wq_sb[dc][:], wq[dc])
                lo, w = LOS[0]
                Q3[(dc + 1) % 3].dma_start(
                    xtiles_s[0][dc][0][:, 0:w], xt[dc][:, lo : lo + w]
                )
            for li in (1, 2):
                lo, w = LOS[li]
                for dc in range(DC):
                    Q3[(li + dc) % 3].dma_start(
                        xtiles_s[0][dc][li][:, 0:w], xt[dc][:, lo : lo + w]
                    )
            bq_sb = cp.tile([128, NG], F32)
            nc.scalar.dma_start(bq_sb[:], bq[:])
            wp_sb = cp.tile([128, 2, D], BF16)
            nc.scalar.dma_start(wp_sb[:], wp[:].rearrange("c p f -> p c f"))
            cst_sb = cp.tile([128, 448], BF16)
            nc.scalar.dma_start(cst_sb[:], cst[:])
            i2_sb = cst_sb[:, 0:64]
            m0_sb = cst_sb[:, 64:192]
            m1_sb = cst_sb[:, 192:320]
            ms_sb = cst_sb[:, 320:448]
            for dc in range(DC):
                xts = []
                for li, (lo, w) in enumerate(LOS):
                    xtile = xtp.tile([128, 512], BF16, tag="xs")
                    Q3[(li + dc) % 3].dma_start(
                        xtile[:, 0:w], xs[dc][:, lo : lo + w]
                    )
                    xts.append(xtile)
                xtiles_s[1].append(xts)

            # ---- persistent per-source tensors
            QV, K2, VNA = [], [], []
            for s in range(2):
                qv = bp.tile([128, NG, T], BF16, name=f"qv{s}")
                k2 = bp.tile([64, T], BF16, name=f"k2{s}")
                vna = bp.tile([128, NP, H3, 65], BF16, name=f"vna{s}")
                nc.gpsimd.memset(vna[:, :, :, 64:65], 1.0)
                QV.append(qv)
                K2.append(k2)
                VNA.append(vna)
            aT01 = [bp.tile([128, T], BF16, name=f"a01_{e}") for e in range(2)]
            aT2 = [bp.tile([64, T], BF16, name=f"a2_{e}") for e in range(2)]

            # views --------------------------------------------------------
            def qT(s, h):
                return QV[s][0:64, h, :]

            def kT(s, h):
                return (QV[s][0:64, 3, :], QV[s][0:64, 4, :], K2[s][:, :])[h]

            def vT(s, h):  # partition base 64
                return QV[s][64:128, 1 + h, :]

            # ---- phase 1: qkv projections -> QV
            for s in range(2):
                xtiles = xtiles_s[s]
                for li, (lo, w) in enumerate(LOS):
                    for g in range(NG):
                        pg = work.tile([128, REG_W], F32, tag="work")
                        for dc in range(DC):
                            nc.tensor.matmul(
                                pg[:, 0:w],
                                wq_sb[dc][:, 128 * g : 128 * (g + 1)],
                                xtiles[dc][li][:, 0:w],
                                start=(dc == 0),
                                stop=(dc == DC - 1),
                            )
                        nc.vector.tensor_scalar_add(
                            QV[s][:, g, lo : lo + w], pg[:, 0:w], bq_sb[:, g : g + 1]
                        )
                # realign k2 (group 0 high half) to partition base 0
                nc.sync.dma_start(K2[s][:], QV[s][64:128, 0, :])

                # ---- phase 2: v natural layout (+ones col already DMA'd)
                for h in range(H3):
                    pt = work.tile([128, 2 * REG_W], BF16, tag="work")
                    for tch in range(NP):
                        nc.tensor.transpose(
                            pt[:, 64 * tch : 64 * (tch + 1)],
                            vT(s, h)[:, 128 * tch : 128 * (tch + 1)],
                            i2_sb[64:128, :],
                        )
                    nc.vector.tensor_copy(
                        VNA[s][:, :, h, 0:64],
                        pt[:, 0 : 64 * NP].rearrange("p (tc d) -> p tc d", d=64),
                    )

            # ---- phase 3+4: attention per (branch, head), proj interleaved
            def attn_head(e, h):
                sq = 0 if e == 0 else 1
                O = psO.tile([128, 1280], F32, tag="O")

                # pack score segments into psum regions
                segs = _chunk_plan(e)
                regions = []  # parts: (kind, idx, qoff, loc, w, choff)
                cur, used = [], 0
                for kind, idx, qoff, width in segs:
                    off = 0
                    while off < width:
                        if REG_W - used < 256:
                            regions.append((cur, used))
                            cur, used = [], 0
                        w = min(width - off, REG_W - used)
                        cur.append((kind, idx, qoff + off, used, w, off))
                        used += w
                        off += w
                if cur:
                    regions.append((cur, used))

                target = (aT01[e][0:64, :], aT01[e][64:128, :], aT2[e][:, :])[h]
                last = e == 1 and h == 2

                def norm_bcast():
                    # off-critical-path: DRAM-roundtrip denominator broadcast
                    # (psO is double-buffered, so holding O is free).  The
                    # DVE recip+mult are DEFERRED one head: emitting them now
                    # would stall the in-order DVE queue (recip waits on the
                    # roundtrip) and the next head's masks behind it.
                    den = denp.tile([1, T], F32, tag="den")
                    nc.scalar.activation(
                        den[:], O[64:65, :], mybir.ActivationFunctionType.Copy
                    )
                    scr = scrp.tile([1, T], F32, tag="scr")
                    nc.gpsimd.dma_start(scr[:], den[:])
                    rb = rbp.tile([64, T], F32, tag="rb")
                    nc.gpsimd.dma_start(rb[:], scr[:].to_broadcast([64, T]))
                    Oc, tgt = O, target

                    def finish():
                        rbi = rbip.tile([64, T], F32, tag="rbi")
                        nc.vector.reciprocal_approx_fast(rbi[:], rb[:])
                        nc.vector.tensor_tensor(
                            tgt, Oc[0:64, :], rbi[:], mybir.AluOpType.mult
                        )

                    return finish

                def norm_pe_batched(qw):
                    # fast path for the proj-gating head: PE ones-matmul
                    # broadcast (tensor queue is idle after the last AV).
                    # Emit phase-major: cross-engine semaphore chains follow
                    # program order, so per-chunk emission serializes.
                    hfs = list(range(0, T, qw))
                    dqs = {}
                    for hf in hfs:
                        dq = denq.tile([1, qw], BF16, tag="dq")
                        nc.scalar.activation(
                            dq[:],
                            O[64:65, hf : hf + qw],
                            mybir.ActivationFunctionType.Copy,
                        )
                        dqs[hf] = dq
                    rbqs = {}
                    for hf in hfs:
                        rbq = work.tile([64, REG_W], F32, tag="work")
                        nc.tensor.matmul(
                            rbq[:, 0:qw],
                            ms_sb[0:1, 0:64],
                            dqs[hf][:],
                            start=True,
                            stop=True,
                        )
                        rbqs[hf] = rbq
                    for hf in hfs:
                        rbi = rbip.tile([64, qw], F32, tag="rbiq")
                        nc.vector.reciprocal_approx_fast(rbi[:], rbqs[hf][:, 0:qw])
                        nc.vector.tensor_tensor(
                            target[:, hf : hf + qw],
                            O[0:64, hf : hf + qw],
                            rbi[:],
                            mybir.AluOpType.mult,
                        )

                def emit_scores(parts, used):
                    rt = work.tile([128, REG_W], F32, tag="work")
                    et = expp.tile([128, REG_W], BF16, tag="expT")
                    for kind, idx, qo, loc, w, choff in parts:
                        kv = kT(sq, h) if kind == "s" else kT(0, h)
                        lhsT = kv[:, 128 * idx : 128 * (idx + 1)]
                        rhs = qT(sq, h)[:, qo : qo + w]
                        # split at psum bank boundaries (512 within rt)
                        p0 = 0
                        while p0 < w:
                            bw = min(w - p0, 512 - ((loc + p0) % 512))
                            nc.tensor.matmul(
                                rt[:, loc + p0 : loc + p0 + bw],
                                lhsT,
                                rhs[:, p0 : p0 + bw],
                                start=True,
                                stop=True,
                            )
                            p0 += bw
                    nc.scalar.activation(
                        et[:, 0:used],
                        rt[:, 0:used],
                        mybir.ActivationFunctionType.Exp,
                        bias=0.0,
                        scale=float(SCALE),
                    )
                    # causal corner fixes (chunk-local cols 0:128 / 0:64)
                    for kind, idx, qo, loc, w, choff in parts:
                        if kind == "s":
                            msk, mw, mo = ms_sb, 128, 0
                        elif e == 0:
                            msk, mw, mo = m0_sb, 128, 0
                        elif idx == 0:
                            msk, mw, mo = m1_sb, 128, 0
                        else:
                            # narrow e=1 trunk chunk: corner is block 2c+1
                            # queries only -> the 64:128 half of m1
                            msk, mw, mo = m1_sb, 64, 64
                        if choff < mw:
                            cw = min(mw - choff, w)
                            nc.vector.tensor_tensor(
                                et[:, loc : loc + cw],
                                et[:, loc : loc + cw],
                                msk[:, mo + choff : mo + choff + cw],
                                mybir.AluOpType.mult,
                            )
                    return et

                def emit_av(parts, et):
                    # AV accumulate into O; trunk c=0 is the unique first
                    # touch (one start=True per psum bank)
                    for kind, idx, qo, loc, w, choff in parts:
                        first = kind == "t" and idx == 0
                        vsrc = VNA[sq] if kind == "s" else VNA[0]
                        lhsT = vsrc[:, idx, h, :]
                        p0 = 0
                        while p0 < w:
                            q0 = qo + p0
                            bw = min(w - p0, 512 - (q0 % 512))
                            nc.tensor.matmul(
                                O[0:65, q0 : q0 + bw],
                                lhsT,
                                et[:, loc + p0 : loc + p0 + bw],
                                start=first,
                                stop=False,
                                skip_group_check=True,
                            )
                            p0 += bw

                # software-pipeline by one region: the tensor queue runs
                # scores r+1 while ACT computes exp r, so AV r never waits
                pend = None
                for parts, used in regions:
                    et = emit_scores(parts, used)
                    if pend is not None:
                        emit_av(*pend)
                    pend = (parts, et)
                emit_av(*pend)
                # flush the previous head's deferred normalize (its
                # broadcast landed while this head was computing)
                while pending_norm:
                    pending_norm.pop(0)()
                if last:
                    norm_pe_batched(320)
                else:
                    pending_norm.append(norm_bcast())

            def proj(e):
                for m in range(NP):
                    ot = outst.tile([128, D], BF16, tag="ot")
                    for lo, w in ((0, 512), (512, 256)):
                        pp = work.tile([128, REG_W], F32, tag="work")
                        nc.tensor.matmul(
                            pp[:, 0:w],
                            aT01[e][:, 128 * m : 128 * (m + 1)],
                            wp_sb[:, 0, lo : lo + w],
                            start=True,
                            stop=False,
                        )
                        nc.tensor.matmul(
                            pp[:, 0:w],
                            aT2[e][:, 128 * m : 128 * (m + 1)],
                            wp_sb[0:64, 1, lo : lo + w],
                            start=False,
                            stop=True,
                        )
                        nc.vector.tensor_copy(ot[:, lo : lo + w], pp[:, 0:w])
                    nc.sync.dma_start(
                        outs[e][128 * m : 128 * (m + 1), :], ot[:]
                    )

            pending_norm = []
            attn_head(0, 0)
            attn_head(0, 1)
            attn_head(0, 2)
            attn_head(1, 0)
            proj(0)
            attn_head(1, 1)
            attn_head(1, 2)
            proj(1)

    nc.finalize()
    return nc


# ---------------------------------------------------------------- host side

_NC = None


def _get_nc():
    global _NC
    if _NC is None:
        _NC = build()
    return _NC


def _consts():
    i2 = np.zeros((128, 64), np.float32)
    i2[:64] = np.eye(64, dtype=np.float32)
    i2[64:] = np.eye(64, dtype=np.float32)
    p = np.arange(128)[:, None]
    x = np.arange(128)[None, :]
    m0 = (p <= x).astype(np.float32)
    ms = np.where(x < 64, p <= x, (p >= 64) & (p <= x)).astype(np.float32)
    m1 = np.zeros((128, 128), np.float32)
    m1[0:64, 64:128] = 1.0
    cst = np.concatenate([i2, m0, m1, ms], axis=1)
    return dict(cst=cst.astype(BF))


def _core_inputs(x0, x1, w_attn, b_attn, w_proj, consts):
    """Build the 8 per-core input maps. Core order: (b, G) row-major."""
    maps = []
    xT = [
        [x[b].reshape(T, D).T.astype(BF).reshape(DC, 128, T) for b in range(B)]
        for x in (x0, x1)
    ]
    for b in range(B):
        for G in range(4):
            gh = [3 * G + h for h in range(H3)]
            qc = [768 + g * 64 + np.arange(64) for g in gh]
            kc = [1536 + g * 64 + np.arange(64) for g in gh]
            vc = [0 + g * 64 + np.arange(64) for g in gh]
            groups = [
                np.concatenate([qc[0], kc[2]]),
                np.concatenate([qc[1], vc[0]]),
                np.concatenate([qc[2], vc[1]]),
                np.concatenate([kc[0], vc[2]]),
                np.concatenate([kc[1], kc[1]]),  # pad half unused
            ]
            cols = np.concatenate(groups)
            wqm = w_attn[:, cols].copy()
            wqm[:, 4 * 128 + 64 :] = 0.0
            bqm = b_attn[cols].reshape(NG, 128).T.copy()
            bqm[64:, 4] = 0.0
            wpm = np.zeros((2, 128, D), np.float32)
            wpm[0] = w_proj[3 * G * 64 : 3 * G * 64 + 128]
            wpm[1, 0:64] = w_proj[3 * G * 64 + 128 : 3 * G * 64 + 192]
            maps.append(
                dict(
                    xt=xT[0][b],
                    xs=xT[1][b],
                    wq=wqm.astype(BF).reshape(DC, 128, NG * 128),
                    bq=np.ascontiguousarray(bqm, np.float32),
                    wp=wpm.astype(BF),
                    **consts,
                )
            )
    return maps


def kernel(x0, x1, w_attn, b_attn, w_proj, b_proj, _trace=False):
    x0 = np.asarray(x0, np.float32)
    x1 = np.asarray(x1, np.float32)
    w_attn = np.asarray(w_attn, np.float32)
    b_attn = np.asarray(b_attn, np.float32)
    w_proj = np.asarray(w_proj, np.float32)
    b_proj = np.asarray(b_proj, np.float32)

    nc = _get_nc()
    maps = _core_inputs(x0, x1, w_attn, b_attn, w_proj, _consts())
    if _trace:
        res = run_bass_kernel_spmd(
            nc, maps, core_ids=list(range(8)), trace=True
        )
    else:
        # an ambient BASS_TRACE=1 would route run_bass_kernel_spmd into the
        # NTFF path, which crashes on this image (antenv.axon_hooks is
        # missing) — pin the non-trace path for the plain call
        import os

        prev = os.environ.get("BASS_NEVER_TRACE")
        os.environ["BASS_NEVER_TRACE"] = "1"
        try:
            res = run_bass_kernel_spmd(
                nc, maps, core_ids=list(range(8)), trace=False
            )
        finally:
            if prev is None:
                os.environ.pop("BASS_NEVER_TRACE", None)
            else:
                os.environ["BASS_NEVER_TRACE"] = prev

    out = [np.zeros((B, T, D), np.float32) for _ in range(2)]
    for ci, r in enumerate(res.results):
        b = ci // 4
        out[0][b] += np.asarray(r["o0"], np.float32)
        out[1][b] += np.asarray(r["o1"], np.float32)
    out0 = (out[0] + b_proj).reshape(B, NB, BS, D)
    out1 = (out[1] + b_proj).reshape(B, NB, BS, D)
    if _trace:
        kernel._last = res
    return out0, out1


if __name__ == "__main__":
    rng = np.random.default_rng(0)
    x0 = rng.standard_normal((B, NB, BS, D), dtype=np.float32)
    x1 = rng.standard_normal((B, NB, BS, D), dtype=np.float32)
    wa = rng.standard_normal((D, 3 * D), dtype=np.float32) * 0.02
    ba = np.zeros(3 * D, np.float32)
    wpj = rng.standard_normal((D, D), dtype=np.float32) * 0.02
    bp_ = np.zeros(D, np.float32)
    o0, o1 = kernel(x0, x1, wa, ba, wpj, bp_)
    print("ran", o0.shape, o1.shape, float(np.abs(o0).mean()))
